# revision 1
# baseline (speedup 1.0000x reference)
"""Trainium2 Bass kernel for 8-head MHA (B=2, S=2048, d_model=512).

Sharding: core c -> batch b = c//4, head-pair hp = c%4 (heads 2hp, 2hp+1).
Each core computes q/k/v projections for its 128 out-dims (2 heads), the
masked-softmax attention for those heads, and the output-projection partial
for its 128 in-dims (heads summed on device). Host sums the 4 partials per
batch and adds the output bias.

On-chip layout is fully "transposed" (feature dims on partitions, sequence
on the free dim) so no activation transposes are ever needed:
  qbT/kbT [128=2*dk, S]            head-pair projections, bf16
  scores^T tiles [128=k-chunk, q]  PSUM f32
  p = exp(s)  (no max subtraction: |s| <= ~4), masked multiplicatively
  PV via v_aug = [v | ones] -> xaug^T [65, q] with row 64 = softmax denom
  normalize: DVE reciprocal + gpsimd partition_broadcast + DVE mul
  o-proj: out_nat [128=q-block, 512] = xhat-slice.T @ WoT-slice
"""

import os
import sys
import types
import numpy as np
import ml_dtypes

HEAD = 8
D = 512
DK = 64
B = 2
N_CORES = 8
P = 128

_NC_CACHE = {}
LAST_RESULTS = None  # test harness reads BassKernelResults from here


def _register_ntff_hook():
    """Make run_bass_kernel_spmd(trace=True) work under axon by registering
    the NTFF profile hook that the trimmed antenv package lacks."""
    if "antenv.axon_hooks" in sys.modules:
        return
    try:
        import antenv

        mod = types.ModuleType("antenv.axon_hooks")
        _hook = [None]
        mod.set_axon_ntff_profile_hook = lambda h: _hook.__setitem__(0, h)
        mod.get_axon_ntff_profile_hook = lambda: _hook[0]
        sys.modules["antenv.axon_hooks"] = mod
        antenv.axon_hooks = mod
        if "/root/.axon_site" not in sys.path:
            sys.path.insert(0, "/root/.axon_site")
        from trn_agent_boot.trn_boot import _ntff_profile_via_ctypes

        mod.set_axon_ntff_profile_hook(
            _ntff_profile_via_ctypes("/opt/axon/libaxon_pjrt.so")
        )
    except Exception:
        pass  # tracing degrades; execution still works


def _build_nc(S):
    import concourse.tile as tile
    import concourse.mybir as mybir
    from concourse import bacc
    from concourse.bass import ts
    from contextlib import ExitStack

    f32 = mybir.dt.float32
    bf16 = mybir.dt.bfloat16
    AF = mybir.ActivationFunctionType

    KC = S // P       # k chunks (score-tile rows == v s-blocks)
    QT = S // 512     # q tiles of 512
    SW = min(1024, S)  # score psum tile width
    QH = S // SW
    EC = D // P       # embed chunks for projections

    nc = bacc.Bacc("TRN2", target_bir_lowering=False, debug=False,
                   num_devices=N_CORES)

    xqT = nc.dram_tensor("xqT", [D, S], bf16, kind="ExternalInput").ap()
    xkT = nc.dram_tensor("xkT", [D, S], bf16, kind="ExternalInput").ap()
    xvT = nc.dram_tensor("xvT", [D, S], bf16, kind="ExternalInput").ap()
    maskT = nc.dram_tensor("maskT", [S, S], bf16, kind="ExternalInput").ap()
    wqT = nc.dram_tensor("wqT", [D, P], bf16, kind="ExternalInput").ap()
    wkT = nc.dram_tensor("wkT", [D, P], bf16, kind="ExternalInput").ap()
    wvT = nc.dram_tensor("wvT", [D, P], bf16, kind="ExternalInput").ap()
    woT = nc.dram_tensor("woT", [P, D], bf16, kind="ExternalInput").ap()
    bq8 = nc.dram_tensor("bq8", [P, 1], f32, kind="ExternalInput").ap()
    bks = nc.dram_tensor("bks", [P, 1], f32, kind="ExternalInput").ap()
    bv_row = nc.dram_tensor("bv_row", [1, P], bf16, kind="ExternalInput").ap()
    outp = nc.dram_tensor("outp", [S, D], bf16, kind="ExternalOutput").ap()
    debug = os.environ.get("MHA_DEBUG", "0") == "1"
    if debug:
        dbg_qbT = nc.dram_tensor("dbg_qbT", [P, S], bf16, kind="ExternalOutput").ap()
        dbg_kbT = nc.dram_tensor("dbg_kbT", [P, S], bf16, kind="ExternalOutput").ap()
        dbg_v = nc.dram_tensor("dbg_v", [P, (S // P) * 130], bf16, kind="ExternalOutput").ap()
        dbg_p0 = nc.dram_tensor("dbg_p0", [P, S], bf16, kind="ExternalOutput").ap()
        dbg_xt0 = nc.dram_tensor("dbg_xt0", [P, S], bf16, kind="ExternalOutput").ap()
        dbg_xhat = nc.dram_tensor("dbg_xhat", [P, S], bf16, kind="ExternalOutput").ap()

    VG = 130  # v_sb column group: [v_h0(64) | 1 | v_h1(64) | 1]

    with tile.TileContext(nc) as tc, ExitStack() as ctx:
        consts = ctx.enter_context(tc.tile_pool(name="consts", bufs=1))
        resid = ctx.enter_context(tc.tile_pool(name="resid", bufs=1))
        mpool = ctx.enter_context(tc.tile_pool(name="maskp", bufs=KC))
        ppool = ctx.enter_context(tc.tile_pool(name="pp", bufs=KC + 1))
        rpool = ctx.enter_context(tc.tile_pool(name="recipp", bufs=2))
        opool = ctx.enter_context(tc.tile_pool(name="outsb", bufs=2))

        # ---- constants / weights / residents ----
        ones_row = consts.tile([1, P], bf16)
        nc.vector.memset(ones_row[:], 1.0)
        ones65 = consts.tile([DK + 1, DK], bf16)
        nc.vector.memset(ones65[:], 1.0)
        bq8_sb = consts.tile([P, 1], f32)
        nc.sync.dma_start(bq8_sb[:], bq8[:])
        bks_sb = consts.tile([P, 1], f32)
        nc.sync.dma_start(bks_sb[:], bks[:])
        bvr_sb = consts.tile([1, P], bf16)
        nc.sync.dma_start(bvr_sb[:], bv_row[:])

        wq_sb = consts.tile([P, D], bf16)  # [p=e%128, ec*128+dk]
        wk_sb = consts.tile([P, D], bf16)
        wv_sb = consts.tile([P, D], bf16)
        for ec in range(D // P):
            nc.sync.dma_start(wq_sb[:, ts(ec, P)], wqT[ec * P:(ec + 1) * P, :])
            nc.sync.dma_start(wk_sb[:, ts(ec, P)], wkT[ec * P:(ec + 1) * P, :])
            nc.sync.dma_start(wv_sb[:, ts(ec, P)], wvT[ec * P:(ec + 1) * P, :])
        wo_sb = consts.tile([P, D], bf16)
        nc.sync.dma_start(wo_sb[:], woT[:])

        qbT = resid.tile([P, S], bf16)
        kbT = resid.tile([P, S], bf16)
        v_sb = resid.tile([P, KC * VG], bf16)
        nc.vector.memset(v_sb[:], 1.0)  # pre-set the ones columns
        xhat = resid.tile([P, S], bf16)
        xtld0 = resid.tile([DK + 1, S], bf16)
        xtld1 = resid.tile([DK + 1, S], bf16)
        recsb = resid.tile([DK, 512], f32)

        mask_t = []

        def emit_mask_dma(kc):
            mt = mpool.tile([P, S], bf16, tag="mask", name=f"mask{kc}")
            nc.sync.dma_start(mt[:], maskT[kc * P:(kc + 1) * P, :])
            mask_t.append(mt)

        # ---- q / v / k projections (v in its own PSUM banks) ----
        with tc.tile_pool(name="proj_ps", bufs=4, space="PSUM") as proj_ps, \
             tc.tile_pool(name="vproj_ps", bufs=4, space="PSUM") as vproj_ps, \
             tc.tile_pool(name="xvpool", bufs=EC) as xvpool, \
             tc.tile_pool(name="xs_pool", bufs=2) as xs_pool:

            def emit_qk_proj(w_sb, bias_sb, scale, dstT, srcT):
                pss = [proj_ps.tile([P, 512], f32, tag="proj", name=f"pj{st}")
                       for st in range(QT)]
                for ec in range(EC):
                    xt = xs_pool.tile([P, S], bf16, tag="xs")
                    nc.sync.dma_start(xt[:], srcT[ec * P:(ec + 1) * P, :])
                    for st in range(QT):
                        nc.tensor.matmul(
                            pss[st][:], w_sb[:, ts(ec, P)],
                            xt[:, ts(st, 512)],
                            start=(ec == 0), stop=(ec == EC - 1),
                        )
                for st in range(QT):
                    nc.scalar.activation(dstT[:, ts(st, 512)], pss[st][:],
                                         AF.Identity, bias=bias_sb[:],
                                         scale=scale)

            xv_t = []
            for ec in range(EC):
                xvt = xvpool.tile([P, S], bf16, tag="xv", name=f"xv{ec}")
                nc.sync.dma_start(xvt[:], xvT[ec * P:(ec + 1) * P, :])
                xv_t.append(xvt)

            def emit_vproj_block(sb):
                vpt = vproj_ps.tile([P, P], f32, tag="vp", name=f"vp{sb}")
                for ec in range(EC):
                    nc.tensor.matmul(vpt[:], xv_t[ec][:, ts(sb, P)],
                                     wv_sb[:, ts(ec, P)],
                                     start=(ec == 0), stop=False)
                nc.tensor.matmul(vpt[:], ones_row[:], bvr_sb[:],
                                 start=False, stop=True)
                # evict into the [v_h0 | 1 | v_h1 | 1] interleaved layout
                nc.vector.tensor_copy(v_sb[:, sb * VG: sb * VG + DK],
                                      vpt[:, 0:DK])
                nc.vector.tensor_copy(
                    v_sb[:, sb * VG + DK + 1: sb * VG + 2 * DK + 1],
                    vpt[:, DK:2 * DK])

            emit_qk_proj(wq_sb, bq8_sb, 0.125, qbT, xqT)
            for sb in range(KC):
                emit_vproj_block(sb)
            emit_qk_proj(wk_sb, bks_sb, 1.0, kbT, xkT)

        # ---- attention ----
        scores_ps = ctx.enter_context(
            tc.tile_pool(name="scores_ps", bufs=2, space="PSUM"))
        xaug_ps = ctx.enter_context(
            tc.tile_pool(name="xaug_ps", bufs=QT, space="PSUM"))

        p_h = {0: [], 1: []}

        def emit_scores(h, kc):
            pt = ppool.tile([P, S], bf16, tag="p")
            for qh in range(QH):
                sc = scores_ps.tile([P, SW], f32, tag="scores")
                for q2 in range(SW // 512):
                    qt = qh * (SW // 512) + q2
                    nc.tensor.matmul(
                        sc[:, ts(q2, 512)],
                        kbT[h * DK:(h + 1) * DK, ts(kc, P)],
                        qbT[h * DK:(h + 1) * DK, ts(qt, 512)],
                        start=True, stop=True,
                    )
                nc.scalar.activation(pt[:, ts(qh, SW)], sc[:], AF.Exp)
            nc.vector.tensor_mul(pt[:], pt[:], mask_t[kc][:])
            if debug and h == 0 and kc == 0:
                nc.sync.dma_start(dbg_p0[:], pt[:])
            p_h[h].append(pt)

        def emit_pv(h, kc, xaugs):
            base = 0 if h == 0 else DK + 1
            for qt in range(QT):
                nc.tensor.matmul(
                    xaugs[qt][:],
                    v_sb[:, kc * VG + base: kc * VG + base + DK + 1],
                    p_h[h][kc][:, ts(qt, 512)],
                    start=(kc == 0), stop=(kc == KC - 1),
                )

        def emit_xevict(h, xaugs):
            dst = xtld0 if h == 0 else xtld1
            for qt in range(QT):
                nc.vector.tensor_copy(dst[:, ts(qt, 512)], xaugs[qt][:])

        def emit_norm_qt(h, qt):
            xtld = xtld0 if h == 0 else xtld1
            nps = scores_ps.tile([DK, 512], f32, tag="scores",
                                 name=f"nps{h}_{qt}")
            nc.tensor.matmul(nps[:], ones65[DK:DK + 1, :],
                             xtld[DK:DK + 1, ts(qt, 512)],
                             start=True, stop=True)
            nc.vector.reciprocal_approx_fast(out=recsb[:], in_=nps[:])
            nc.vector.tensor_mul(
                xhat[h * DK:(h + 1) * DK, ts(qt, 512)],
                xtld[0:DK, ts(qt, 512)], recsb[:])

        # phase A0: head-0 scores (mask chunks fetched just-in-time)
        for kc in range(KC):
            emit_mask_dma(kc)
            emit_scores(0, kc)

        # phase A1 // B0
        xaug0 = [xaug_ps.tile([DK + 1, 512], f32, tag="xaug",
                              name=f"xg0_{qt}") for qt in range(QT)]
        for kc in range(KC):
            emit_scores(1, kc)
            emit_pv(0, kc, xaug0)
        emit_xevict(0, xaug0)

        # phase B1 (+ norm0 overlapped by scheduler)
        xaug1 = [xaug_ps.tile([DK + 1, 512], f32, tag="xaug",
                              name=f"xg1_{qt}") for qt in range(QT)]
        for qt in range(QT):
            emit_norm_qt(0, qt)
        for kc in range(KC):
            emit_pv(1, kc, xaug1)
        emit_xevict(1, xaug1)

        if debug:
            nc.sync.dma_start(dbg_qbT[:], qbT[:])
            nc.sync.dma_start(dbg_kbT[:], kbT[:])
            nc.sync.dma_start(dbg_v[:], v_sb[:])
            nc.sync.dma_start(dbg_xt0[0:DK + 1, :], xtld0[:])
            nc.sync.dma_start(dbg_xhat[:], xhat[:])

        # ---- norm1 + output projection, pipelined per q-tile ----
        for qt in range(QT):
            emit_norm_qt(1, qt)
            for qb in range(qt * 4, qt * 4 + 4):
                op = xaug_ps.tile([P, 512], f32, tag="xaug", name=f"op{qb}")
                nc.tensor.matmul(op[:], xhat[:, ts(qb, P)], wo_sb[:],
                                 start=True, stop=True)
                ob = opool.tile([P, D], bf16, tag="ob")
                nc.vector.tensor_copy(ob[:], op[:])
                nc.sync.dma_start(outp[qb * P:(qb + 1) * P, :], ob[:])

    nc.compile()
    return nc


def _get_nc(S):
    if S not in _NC_CACHE:
        _NC_CACHE[S] = _build_nc(S)
    return _NC_CACHE[S]


def kernel(query, key, value, mask, Wq, bq, Wk, bk, Wv, bv, Wo, bo):
    global LAST_RESULTS
    trace = os.environ.get("MHA_TRACE", "0") == "1"
    if trace:
        _register_ntff_hook()

    from concourse.bass_utils import run_bass_kernel_spmd

    query = np.asarray(query)
    key = np.asarray(key)
    value = np.asarray(value)
    mask = np.asarray(mask)
    Wq, bq, Wk, bk = map(np.asarray, (Wq, bq, Wk, bk))
    Wv, bv, Wo, bo = map(np.asarray, (Wv, bv, Wo, bo))

    S = query.shape[1]
    nc = _get_nc(S)

    bf = ml_dtypes.bfloat16
    maskTb = np.ascontiguousarray((mask[0] != 0).T).astype(bf)
    xT = {}
    for b in range(B):
        xT[("q", b)] = np.ascontiguousarray(query[b].T).astype(bf)
        xT[("k", b)] = np.ascontiguousarray(key[b].T).astype(bf)
        xT[("v", b)] = np.ascontiguousarray(value[b].T).astype(bf)

    in_maps = []
    for c in range(N_CORES):
        b, hp = divmod(c, 4)
        sl = slice(P * hp, P * hp + P)
        in_maps.append({
            "xqT": xT[("q", b)],
            "xkT": xT[("k", b)],
            "xvT": xT[("v", b)],
            "maskT": maskTb,
            "wqT": np.ascontiguousarray(Wq[sl, :].T).astype(bf),
            "wkT": np.ascontiguousarray(Wk[sl, :].T).astype(bf),
            "wvT": np.ascontiguousarray(Wv[sl, :].T).astype(bf),
            "woT": np.ascontiguousarray(Wo[:, sl].T).astype(bf),
            "bq8": (bq[sl] / 8.0).reshape(P, 1).astype(np.float32),
            "bks": bk[sl].reshape(P, 1).astype(np.float32),
            "bv_row": bv[sl].reshape(1, P).astype(bf),
        })

    res = run_bass_kernel_spmd(
        nc, in_maps, core_ids=list(range(N_CORES)),
        trace=trace, trace_cores=[0] if trace else None,
    )
    LAST_RESULTS = res

    out = np.zeros((B, S, D), np.float32)
    for c in range(N_CORES):
        out[c // 4] += res.results[c]["outp"].astype(np.float32)
    out += bo.astype(np.float32)
    return out



# revision 11
# speedup vs baseline: 1.3660x; 1.3660x over previous
"""Trainium2 Bass kernel for 8-head MHA (B=2, S=2048, d_model=512) — v4.

Sharding: core c -> batch b = c//4, head-pair hp = c%4 (heads 2hp, 2hp+1).

Key facts driving the design (measured on this part):
  - PE is HAM-throttled to 4/8 clock (1.2 GHz) after ~10us of sustained
    activity: bf16 streams ~1 row/ns. fp8 DoubleRow streams 2 contraction
    rows/cycle at +13%/instr, so it only pays where it halves the
    instruction count (contraction >= 256): that is PV only.
  - ACT exp costs free_size x 0.83ns regardless of partitions/dtype.
  - The mask is applied with ZERO engine time: gpsimd software-DGE DMAs
    with accum_op=bitwise_and AND fp8 p against 0x00/0xFF bytes in DRAM.

Pipeline (bf16 unless noted):
  P:  q/k/v projections (baseline math); v evicted to fp8 v_sb [128,KC,80]
      (64 v-cols + ones col + pad), q with 1/8 folded.
  A:  head-0 scores (K=64 matmuls into [128,1024] psum) -> exp -> fp8
      p_res[h] [128,KC,S]; after each 4-chunk group, gpsimd AND-DMA masks.
  B:  head-1 scores/exp interleaved with head-0 PV (fp8 DoubleRow,
      K=256/instr: v_sb [128,2,80] x p_res [128,2,512] -> xaug[80,512]).
  C:  head-1 PV + head-0 norm (denom bcast matmul + DVE recip + mul).
  D:  norm1 + output projection + bf16 out DMA.

Host sums the 4 partial outputs per batch and adds bo.
"""

import os
import sys
import types
import numpy as np
import ml_dtypes

HEAD = 8
D = 512
DK = 64
B = 2
N_CORES = 8
P = 128

_NC_CACHE = {}
LAST_RESULTS = None  # test harness reads BassKernelResults from here


def _register_ntff_hook():
    """Make run_bass_kernel_spmd(trace=True) work under axon by registering
    the NTFF profile hook that the trimmed antenv package lacks."""
    if "antenv.axon_hooks" in sys.modules:
        return
    try:
        import antenv

        mod = types.ModuleType("antenv.axon_hooks")
        _hook = [None]
        mod.set_axon_ntff_profile_hook = lambda h: _hook.__setitem__(0, h)
        mod.get_axon_ntff_profile_hook = lambda: _hook[0]
        sys.modules["antenv.axon_hooks"] = mod
        antenv.axon_hooks = mod
        if "/root/.axon_site" not in sys.path:
            sys.path.insert(0, "/root/.axon_site")
        from trn_agent_boot.trn_boot import _ntff_profile_via_ctypes

        mod.set_axon_ntff_profile_hook(
            _ntff_profile_via_ctypes("/opt/axon/libaxon_pjrt.so")
        )
    except Exception:
        pass  # tracing degrades; execution still works


def _build_nc(S):
    import concourse.tile as tile
    import concourse.mybir as mybir
    from concourse import bacc
    from concourse.bass import ts
    from contextlib import ExitStack

    f32 = mybir.dt.float32
    bf16 = mybir.dt.bfloat16
    fp8 = mybir.dt.float8e4
    AF = mybir.ActivationFunctionType
    DR = mybir.MatmulPerfMode.DoubleRow
    AND = mybir.AluOpType.bitwise_and

    KC = S // P        # 16 kpos chunks of 128
    QT = S // 512      # 4 q tiles of 512
    PC = KC // 2       # 8 chunk-pairs for PV
    VP = 80            # PV stationary pitch (%16==0, even M)
    MG = 4             # mask-AND group: chunks per gpsimd DMA

    nc = bacc.Bacc("TRN2", target_bir_lowering=False, debug=False,
                   num_devices=N_CORES)

    xq = nc.dram_tensor("xq", [P, 4, S], bf16, kind="ExternalInput").ap()
    xk = nc.dram_tensor("xk", [P, 4, S], bf16, kind="ExternalInput").ap()
    xv = nc.dram_tensor("xv", [P, 4, S], bf16, kind="ExternalInput").ap()
    wq = nc.dram_tensor("wq", [P, 4, P], bf16, kind="ExternalInput").ap()
    wk = nc.dram_tensor("wk", [P, 4, P], bf16, kind="ExternalInput").ap()
    wv = nc.dram_tensor("wv", [P, 4, P], bf16, kind="ExternalInput").ap()
    wo = nc.dram_tensor("wo", [P, D], bf16, kind="ExternalInput").ap()
    u16 = mybir.dt.uint16
    mask16 = nc.dram_tensor("mask16", [P, KC, S // 2], u16,
                            kind="ExternalInput").ap()
    bq8 = nc.dram_tensor("bq8", [P, 1], f32, kind="ExternalInput").ap()
    bks = nc.dram_tensor("bks", [P, 1], f32, kind="ExternalInput").ap()
    bvr = nc.dram_tensor("bvr", [1, P], bf16, kind="ExternalInput").ap()
    outp = nc.dram_tensor("outp", [S, D], bf16, kind="ExternalOutput").ap()

    with tile.TileContext(nc) as tc, ExitStack() as ctx:
        consts = ctx.enter_context(tc.tile_pool(name="consts", bufs=1))
        resid = ctx.enter_context(tc.tile_pool(name="resid", bufs=1))
        opool = ctx.enter_context(tc.tile_pool(name="outsb", bufs=4))
        mpool = ctx.enter_context(tc.tile_pool(name="maskp", bufs=3))

        # ---- weights / inputs (few big DMAs, critical-path first) ----
        wq_sb = consts.tile([P, 4, P], bf16)
        nc.sync.dma_start(wq_sb[:], wq[:])
        xq_sb = consts.tile([P, 4, S], bf16)
        nc.sync.dma_start(xq_sb[:], xq[:])
        wk_sb = consts.tile([P, 4, P], bf16)
        nc.sync.dma_start(wk_sb[:], wk[:])
        xk_sb = consts.tile([P, 4, S], bf16)
        nc.sync.dma_start(xk_sb[:], xk[:])
        wv_sb = consts.tile([P, 4, P], bf16)
        nc.sync.dma_start(wv_sb[:], wv[:])
        xv_sb = consts.tile([P, 4, S], bf16)
        nc.sync.dma_start(xv_sb[:], xv[:])
        wo_sb = consts.tile([P, D], bf16)
        nc.sync.dma_start(wo_sb[:], wo[:])
        bq_sb = consts.tile([P, 1], f32)
        nc.sync.dma_start(bq_sb[:], bq8[:])
        bk_sb = consts.tile([P, 1], f32)
        nc.sync.dma_start(bk_sb[:], bks[:])
        bvr_sb = consts.tile([1, P], bf16)
        nc.sync.dma_start(bvr_sb[:], bvr[:])
        ones_row = consts.tile([1, P], bf16)
        nc.vector.memset(ones_row[:], 1.0)
        ones65 = consts.tile([DK + 1, DK], bf16)
        nc.vector.memset(ones65[:], 1.0)

        # ---- residents ----
        qbT = resid.tile([P, S], bf16)
        kbT = resid.tile([P, S], bf16)
        v_sb = [resid.tile([P, KC, VP], fp8, name=f"vsb{h}")
                for h in range(2)]
        for h in range(2):
            nc.vector.memset(v_sb[h][:, :, DK:], 0.0)
            nc.vector.memset(v_sb[h][:, :, DK:DK + 1], 1.0)
        p_res = [resid.tile([P, KC, S], fp8, name=f"p{h}") for h in range(2)]
        xtld = [resid.tile([DK + 1, S], bf16, name=f"xt{h}") for h in range(2)]
        xhat = resid.tile([P, S], bf16)
        recsb = resid.tile([DK, 512], f32)

        # ---- phase P: projections (bf16, baseline math) ----
        with tc.tile_pool(name="proj_ps", bufs=4, space="PSUM") as proj_ps, \
             tc.tile_pool(name="vp_ps", bufs=4, space="PSUM") as vp_ps:

            def emit_qk_proj(w_t, x_t, bias_t, scale, dstT):
                for qt in range(QT):
                    ps = proj_ps.tile([P, 512], f32, tag="pj")
                    for ec in range(4):
                        nc.tensor.matmul(
                            ps[:], w_t[:, ec, :], x_t[:, ec, ts(qt, 512)],
                            start=(ec == 0), stop=(ec == 3))
                    nc.scalar.activation(dstT[:, ts(qt, 512)], ps[:],
                                         AF.Identity, bias=bias_t[:],
                                         scale=scale)

            emit_qk_proj(wq_sb, xq_sb, bq_sb, 0.125, qbT)
            emit_qk_proj(wk_sb, xk_sb, bk_sb, 1.0, kbT)

            for sb in range(KC):
                vp = vp_ps.tile([P, P], f32, tag="vp")
                for ec in range(4):
                    nc.tensor.matmul(vp[:], xv_sb[:, ec, ts(sb, P)],
                                     wv_sb[:, ec, :],
                                     start=(ec == 0), stop=False)
                nc.tensor.matmul(vp[:], ones_row[:], bvr_sb[:],
                                 start=False, stop=True)
                for h in range(2):
                    nc.scalar.activation(
                        v_sb[h][:, sb, 0:DK], vp[:, h * DK:(h + 1) * DK],
                        AF.Identity)

        # ---- phases A/B/C ----
        scores_ps = ctx.enter_context(
            tc.tile_pool(name="scores_ps", bufs=2, space="PSUM"))
        xaug_ps = ctx.enter_context(
            tc.tile_pool(name="xaug_ps", bufs=2, space="PSUM"))
        norm_ps = ctx.enter_context(
            tc.tile_pool(name="norm_ps", bufs=2, space="PSUM"))

        def emit_scores(h, kc):
            # unmasked exp -> fp8; mask = DVE bitwise-AND on a uint16 view
            mt = mpool.tile([P, S // 2], u16, tag="mask")
            nc.sync.dma_start(mt[:], mask16[:, kc, :])
            for qh in range(2):
                sc = scores_ps.tile([P, 1024], f32, tag="sc")
                for q2 in range(2):
                    qt = 2 * qh + q2
                    nc.tensor.matmul(
                        sc[:, ts(q2, 512)],
                        kbT[h * DK:(h + 1) * DK, ts(kc, P)],
                        qbT[h * DK:(h + 1) * DK, ts(qt, 512)],
                        start=True, stop=True)
                nc.scalar.activation(p_res[h][:, kc, ts(qh, 1024)], sc[:],
                                     AF.Exp)
            p16 = p_res[h][:, kc, :].bitcast(u16)
            nc.vector.tensor_tensor(out=p16, in0=p16, in1=mt[:], op=AND)

        def emit_pv_qt(h, qt):
            xg = xaug_ps.tile([VP, 512], f32, tag="xaug",
                              name=f"xg{h}_{qt}")
            for pc in range(PC):
                nc.tensor.matmul(
                    xg[:], v_sb[h][:, 2 * pc:2 * pc + 2, :],
                    p_res[h][:, 2 * pc:2 * pc + 2, ts(qt, 512)],
                    start=(pc == 0), stop=(pc == PC - 1), perf_mode=DR)
            nc.vector.tensor_copy(xtld[h][:, ts(qt, 512)], xg[0:DK + 1, :])

        def emit_norm_qt(h, qt):
            nps = norm_ps.tile([DK, 512], f32, tag="nps")
            nc.tensor.matmul(nps[:], ones65[DK:DK + 1, :],
                             xtld[h][DK:DK + 1, ts(qt, 512)],
                             start=True, stop=True)
            nc.vector.reciprocal_approx_fast(out=recsb[:], in_=nps[:])
            nc.vector.tensor_mul(xhat[h * DK:(h + 1) * DK, ts(qt, 512)],
                                 xtld[h][0:DK, ts(qt, 512)], recsb[:])

        # phase A: head-0 scores
        for kc in range(KC):
            emit_scores(0, kc)

        # phase B: head-1 scores // head-0 PV
        for kc in range(KC):
            emit_scores(1, kc)
            if kc % 4 == 3:
                emit_pv_qt(0, kc // 4)

        # phase C: head-1 PV // head-0 norm
        for qt in range(QT):
            emit_norm_qt(0, qt)
            emit_pv_qt(1, qt)

        # ---- phase D: norm1 + output projection, pipelined per q-tile ----
        for qt in range(QT):
            emit_norm_qt(1, qt)
            for qb in range(qt * 4, qt * 4 + 4):
                op = xaug_ps.tile([P, 512], f32, tag="xaug", name=f"op{qb}")
                nc.tensor.matmul(op[:], xhat[:, ts(qb, P)], wo_sb[:],
                                 start=True, stop=True)
                ob = opool.tile([P, D], bf16, tag="ob")
                if qb % 2 == 0:
                    nc.scalar.activation(ob[:], op[:], AF.Copy)
                else:
                    nc.vector.tensor_copy(ob[:], op[:])
                nc.sync.dma_start(outp[qb * P:(qb + 1) * P, :], ob[:])

    nc.compile()
    return nc


def _get_nc(S):
    if S not in _NC_CACHE:
        _NC_CACHE[S] = _build_nc(S)
    return _NC_CACHE[S]


def kernel(query, key, value, mask, Wq, bq, Wk, bk, Wv, bv, Wo, bo):
    global LAST_RESULTS
    trace = os.environ.get("MHA_TRACE", "0") == "1"
    if trace:
        _register_ntff_hook()

    from concourse.bass_utils import run_bass_kernel_spmd

    query = np.asarray(query)
    key = np.asarray(key)
    value = np.asarray(value)
    mask = np.asarray(mask)
    Wq, bq, Wk, bk = map(np.asarray, (Wq, bq, Wk, bk))
    Wv, bv, Wo, bo = map(np.asarray, (Wv, bv, Wo, bo))

    S = query.shape[1]
    KC = S // P
    nc = _get_nc(S)

    bf = ml_dtypes.bfloat16
    f8 = ml_dtypes.float8_e4m3

    def x_pack(x):  # [S, 512] -> [128, 4, S] bf16  (x^T chunked by 128)
        return np.ascontiguousarray(
            x.T.reshape(4, P, S).transpose(1, 0, 2)).astype(bf)

    def w_pack(W, sl):  # [128, 4, 128]: W'[p,c,o] = W[sl_o, c*128+p]
        Wt = W[sl, :].T.reshape(4, P, P).transpose(1, 0, 2)
        return np.ascontiguousarray(Wt).astype(bf)

    # mask bytes 0xFF/0x00 paired little-endian into uint16 lanes
    maskT = (np.asarray(mask[0]) != 0).T  # [kpos, q]
    mBytes = np.where(maskT, 0xFF, 0).astype(np.uint8)
    mM = np.ascontiguousarray(
        mBytes.reshape(KC, P, S).transpose(1, 0, 2)).reshape(P, KC * S)
    mM = mM.view(np.uint16).reshape(P, KC, S // 2)

    xq_b = {b: x_pack(query[b]) for b in range(B)}
    xk_b = {b: x_pack(key[b]) for b in range(B)}
    xv_b = {b: x_pack(value[b]) for b in range(B)}

    in_maps = []
    for c in range(N_CORES):
        b, hp = divmod(c, 4)
        sl = slice(P * hp, P * hp + P)
        in_maps.append({
            "xq": xq_b[b],
            "xk": xk_b[b],
            "xv": xv_b[b],
            "wq": w_pack(Wq, sl),
            "wk": w_pack(Wk, sl),
            "wv": w_pack(Wv, sl),
            "wo": np.ascontiguousarray(Wo[:, sl].T).astype(bf),
            "mask16": mM,
            "bq8": (bq[sl] / 8.0).reshape(P, 1).astype(np.float32),
            "bks": bk[sl].reshape(P, 1).astype(np.float32),
            "bvr": bv[sl].reshape(1, P).astype(bf),
        })

    res = run_bass_kernel_spmd(
        nc, in_maps, core_ids=list(range(N_CORES)),
        trace=trace, trace_cores=[0] if trace else None,
    )
    LAST_RESULTS = res

    out = np.zeros((B, S, D), np.float32)
    for c in range(N_CORES):
        out[c // 4] += res.results[c]["outp"].astype(np.float32)
    out += bo.astype(np.float32)
    return out
